# revision 1
# baseline (speedup 1.0000x reference)
"""Swin-style window-attention encoder as a Bass/Tile kernel for TRN2 — v3.

Key design vs v1:
- Residual master X lives in SBUF as FP32 [128, 4, T+16] (channel-major) —
  the residual stream never rounds to bf16 (bf16 master measured 2.2e-2 max
  rel err, over the 2e-2 budget; f32 master 7.8e-3). Matmul inputs are
  bf16 copies cast on the (otherwise idle) GPSIMD engine; per-token mean
  stats contract the f32 master directly (f32 matmul, tiny N).
- Weights are baked into the NEFF as inline consts — per-launch IO is just
  x (bf16 in) and out (bf16).
- Per-layer phase batching: [attention (Exp table)] -> [LN1 rows (Sqrt) +
  apply] -> [FFN (Relu, no table load)] -> [LN2 rows + apply]. 2 activation
  table loads per layer instead of ~64.
- Attention softmax denominators: collected per head into smat rows (act
  Copy), broadcast to 128 partitions via one e2 matmul, ONE fat [128,144]
  DVE reciprocal per head-pair (no 1-lane recips).
- LN row math on [128,36] shuffled layout (SBUF->SBUF strided DMA), not
  1-lane [1,T] ops.
- O-proj / QK-proj / stats at window-pair (288 tokens) granularity; FFN at
  512-token chunks.
"""
from contextlib import ExitStack

import numpy as np
import ml_dtypes

import concourse.bass as bass
import concourse.bacc as bacc
import concourse.tile as tile
import concourse.mybir as mybir

F32 = mybir.dt.float32
F32R = mybir.dt.float32r
BF16 = mybir.dt.bfloat16
AF = mybir.ActivationFunctionType

WS = 12
N = WS * WS          # 144 tokens per window
C = 512
NH = 8
HD = 64
FF = 2048
EPS = 1e-5


def _insdim_ap(row_ap, stride, num, at=1):
    """Insert a dim of (stride, num) at position `at` of the AP (default:
    right after the partition dim). stride=0 -> broadcast; else gather."""
    dims = [list(d) for d in row_ap.ap]
    return bass.AP(
        tensor=row_ap.tensor,
        offset=row_ap.offset,
        ap=dims[:at] + [[stride, num]] + dims[at:],
    )


def _bcast_ap(row_ap, parts):
    return _insdim_ap(row_ap, 0, parts)


def build(nc: bass.Bass, NW: int, NL: int, w: dict,
          skip_attn=False, skip_ffn=False, skip_heads=False,
          pb=(4, 4), winb=2, epb=3, sqb=1, hbb=1, bcb=1, scb=2, la=3, xbb=1):
    """w: packed numpy weight dict (see pack_weights)."""
    T = NW * N
    PAIRS = NW // 2
    NCH = T // 512
    assert T % 512 == 0

    d = {}
    d["x"] = nc.dram_tensor("x", [128, 4, T], BF16, kind="ExternalInput").ap()
    d["out"] = nc.dram_tensor("out", [128, 4, T], BF16, kind="ExternalOutput").ap()
    cst = {nm: nc.inline_tensor(arr, name=nm).ap() for nm, arr in w.items()}

    with tile.TileContext(nc) as tc, ExitStack() as ctx:
        P = lambda name, bufs, **kw: ctx.enter_context(
            tc.tile_pool(name=name, bufs=bufs, **kw)
        )
        xp = P("xmaster", 1)
        cons = P("consts", 1)
        wpA = P("wtsA", 1)     # attention-phase weights
        wpF = P("wtsF", 1)     # ffn-phase weights
        winp = P("win", winb)  # per-pair working tiles
        ep = P("eptiles", epb)  # P tiles
        etp = P("ettiles", 2)   # exp tiles (short-lived)
        sqp = P("sqtiles", sqb)  # squared-x tiles for stats
        scp = P("sctiles", scb)  # recip rows [128,144]
        rowp = P("rows", 1)    # stat rows (persist per-LN)
        lnp = P("lnwork", 2)   # LN shuffle tiles
        bcp = P("bcast", bcb)  # broadcast destinations
        hp = P("hbuf", hbb)
        xbp = P("xbcast", xbb)
        xb2p = P("xb2cast", 1)
        psmm = P("psmm", pb[0], space="PSUM")
        psaux = P("psaux", pb[1], space="PSUM")

        # ---- persistent tiles ----
        X = xp.tile([128, 4, T + 16], F32, tag="X")
        for tq in range(NCH):
            xin = sqp.tile([128, 4, 512], BF16, tag="xsq2")
            nc.sync.dma_start(out=xin,
                              in_=d["x"][:, :, tq * 512:(tq + 1) * 512])
            nc.vector.tensor_copy(out=X[:, :, tq * 512:(tq + 1) * 512], in_=xin)
        ones = cons.tile([128, 1], BF16, tag="ones")       # value 1/512
        nc.sync.dma_start(out=ones, in_=cst["c_ones"])
        onesr = cons.tile([1, 512], BF16, tag="onesr")
        nc.sync.dma_start(out=onesr, in_=cst["c_onesrow"])
        e2 = cons.tile([64, 128], BF16, tag="e2")
        nc.sync.dma_start(out=e2, in_=cst["c_e2"])
        eps128 = cons.tile([128, 1], F32, tag="eps128")
        nc.vector.memset(eps128, EPS)
        onesf = cons.tile([128, 1], F32, tag="onesf")
        nc.vector.memset(onesf, 1.0 / 512.0)
        smats = [cons.tile([64, 144], BF16, tag=f"smat{i}", name=f"smat{i}")
                 for i in range(8)]
        for t in smats:
            nc.vector.memset(t, 0.0)
        mrow = rowp.tile([1, 2 * T], BF16, tag="mrow")      # [mean | meansq]

        for l in range(NL):
            # layer weights (attention set + rows)
            wq = wpA.tile([128, 4, 512], BF16, tag="wq")
            wk = wpA.tile([128, 4, 512], BF16, tag="wk")
            wv = wpA.tile([128, 4, 512], BF16, tag="wv")
            wo = wpA.tile([128, 4, 512], BF16, tag="wo")
            eb = wpA.tile([128, NH, 288], BF16, tag="expb")
            bq = wpA.tile([128, 4], F32, tag="bq")
            bk = wpA.tile([128, 4], F32, tag="bk")
            bo = wpA.tile([128, 4], F32, tag="bo")
            bv = wpA.tile([128, 512], BF16, tag="bvb")
            g1 = wpA.tile([128, 4], F32, tag="g1")
            b1 = wpA.tile([128, 4], F32, tag="b1")
            g2 = wpA.tile([128, 4], F32, tag="g2")
            b2 = wpA.tile([128, 4], F32, tag="b2")
            for nm, t in (("wq", wq), ("wk", wk), ("wv", wv), ("wo", wo),
                          ("expb", eb), ("bq", bq), ("bk", bk), ("bo_c", bo),
                          ("bvb", bv), ("g1", g1), ("b1", b1), ("g2", g2),
                          ("b2", b2)):
                nc.sync.dma_start(out=t, in_=cst[nm][l])
            # ffn weights: issued now, consumed after LN1 (overlaps attention)
            w1 = wpF.tile([128, 4, FF], BF16, tag="w1")
            w2 = wpF.tile([128, 16, 512], BF16, tag="w2")
            bf1 = wpF.tile([128, 16], F32, tag="bf1")
            bf2 = wpF.tile([128, 4], F32, tag="bf2")
            for nm, t in (("w1", w1), ("w2", w2), ("bf1", bf1), ("bf2_c", bf2)):
                nc.sync.dma_start(out=t, in_=cst[nm][l])

            # ---------------- per-pair attention emitter --------------------
            def att_pair(p):
                cs0 = p * 288
                xs = X[:, :, cs0:cs0 + 288]
                xb = xbp.tile([128, 4, 304], BF16, tag="xb")
                nc.gpsimd.tensor_copy(out=xb, in_=X[:, :, cs0:cs0 + 304])
                # tail tokens of both windows packed at cols {0:16, 32:48}
                xt = xbp.tile([128, 4, 64], BF16, tag="xt")
                nc.gpsimd.tensor_copy(
                    out=_insdim_ap(xt[:, :, 0:16], 32, 2, at=2),
                    in_=_insdim_ap(X[:, :, cs0 + 128:cs0 + 144], 144, 2, at=2))
                qw = winp.tile([128, 4, 288], BF16, tag="qw")
                kw = winp.tile([128, 4, 288], BF16, tag="kw")
                for mc in range(4):
                    pq = psmm.tile([128, 288], F32, tag="mm")
                    for kc in range(4):
                        nc.tensor.matmul(pq, lhsT=wq[:, kc, mc * 128:(mc + 1) * 128],
                                         rhs=xb[:, kc, 0:288], start=(kc == 0), stop=(kc == 3))
                    nc.scalar.activation(out=qw[:, mc, :], in_=pq, func=AF.Identity,
                                         bias=bq[:, mc:mc + 1])
                    pk = psmm.tile([128, 288], F32, tag="mm")
                    for kc in range(4):
                        nc.tensor.matmul(pk, lhsT=wk[:, kc, mc * 128:(mc + 1) * 128],
                                         rhs=xb[:, kc, 0:288], start=(kc == 0), stop=(kc == 3))
                    nc.scalar.activation(out=kw[:, mc, :], in_=pk, func=AF.Identity,
                                         bias=bk[:, mc:mc + 1])

                vws = []
                for wi in (0, 1):
                    vw1 = winp.tile([128, NH, 65], BF16, tag=f"vw1_{wi}")
                    off = wi * 144
                    pv1 = psmm.tile([128, 512], F32, tag="mm")
                    for kc in range(4):
                        nc.tensor.matmul(pv1, lhsT=xb[:, kc, off:off + 128],
                                         rhs=wv[:, kc, :], start=(kc == 0), stop=(kc == 3))
                    nc.vector.tensor_add(out=vw1[:, :, 0:64],
                                         in0=pv1.rearrange("p (h e) -> p h e", h=NH),
                                         in1=bv.rearrange("p (h e) -> p h e", h=NH))
                    nc.vector.memset(vw1[:, :, 64:65], 1.0)
                    vws.append(vw1)
                # merged tail-V for both windows: lhsT cols {128:160, 272:304}
                # -> out partitions A-tail 0:16, (garbage 16:32), B-tail 32:48
                vw2p = winp.tile([64, NH, 65], BF16, tag="vw2p")
                pv2 = psmm.tile([64, 512], F32, tag="mm")
                for kc in range(4):
                    nc.tensor.matmul(pv2, lhsT=xt[:, kc, :], rhs=wv[:, kc, :],
                                     start=(kc == 0), stop=(kc == 3))
                nc.vector.tensor_add(out=vw2p[:, :, 0:64],
                                     in0=pv2.rearrange("p (h e) -> p h e", h=NH),
                                     in1=bv[0:64].rearrange("p (h e) -> p h e", h=NH))
                nc.vector.memset(vw2p[:, :, 64:65], 1.0)

                ocm = winp.tile([128, 4, 288], BF16, tag="ocm")
                if skip_heads:
                    nc.vector.tensor_copy(out=ocm, in_=xs)

                # software-pipelined head loop: stage A (S-mm, exp, P-mul) runs
                # `LOOKAHEAD` heads in front of stage B (PV, den) and stage C
                # (per head-pair: e2 bcast-mm, recip, ocm scale), so the PE has
                # independent matmuls queued while act/DVE chew on earlier heads.
                heads = [(wi, h) for wi in (0, 1) for h in range(NH)]
                pts = {}
                psos = {}

                def stage_a(wi, h):
                    off = wi * 144
                    tb = 32 * wi
                    ro, tl = (h % 2) * 64, h // 2
                    ps_s = psmm.tile([128, 288], F32, tag="mm")
                    nc.tensor.matmul(ps_s[:, 0:144],
                                     lhsT=kw[ro:ro + 64, tl, off:off + 128],
                                     rhs=qw[ro:ro + 64, tl, off:off + 144],
                                     start=True, stop=True)
                    nc.tensor.matmul(ps_s[tb:tb + 16, 144:288],
                                     lhsT=kw[ro:ro + 64, tl, off + 128:off + 144],
                                     rhs=qw[ro:ro + 64, tl, off:off + 144],
                                     start=True, stop=True)
                    et = etp.tile([128, 288], BF16, tag="e")
                    nc.scalar.activation(out=et, in_=ps_s, func=AF.Exp)
                    pt = ep.tile([128, 288], BF16, tag="p")
                    nc.vector.tensor_mul(pt, et, eb[:, h, :])
                    pts[(wi, h)] = pt

                def stage_b(wi, h):
                    pt = pts.pop((wi, h))
                    vw1 = vws[wi]
                    smat = smats[wi * 4 + h // 2]
                    ps_o = psaux.tile([65, 144], F32, tag="aux")
                    nc.tensor.matmul(ps_o, lhsT=vw1[:, h, :], rhs=pt[:, 0:144],
                                     start=True, stop=False)
                    tb = 32 * wi
                    nc.tensor.matmul(ps_o, lhsT=vw2p[tb:tb + 16, h, :],
                                     rhs=pt[tb:tb + 16, 144:288],
                                     start=False, stop=True)
                    nc.scalar.activation(out=smat[32 * (h % 2):32 * (h % 2) + 1, :],
                                         in_=ps_o[64:65, 0:144], func=AF.Copy)
                    psos[(wi, h)] = ps_o
                    if h % 2 == 1:
                        stage_c(wi, h // 2, smat)

                def stage_c(wi, hpair, smat):
                    off = wi * 144
                    ps_sc = psmm.tile([128, 144], F32, tag="mm")
                    nc.tensor.matmul(ps_sc, lhsT=e2, rhs=smat, start=True, stop=True)
                    sc = scp.tile([128, 144], F32, tag="scsb")
                    nc.vector.reciprocal(out=sc, in_=ps_sc)
                    p0 = psos.pop((wi, 2 * hpair))
                    p1 = psos.pop((wi, 2 * hpair + 1))
                    nc.vector.tensor_mul(ocm[0:64, hpair, off:off + 144],
                                         p0[0:64, :], sc[0:64, :])
                    nc.vector.tensor_mul(ocm[64:128, hpair, off:off + 144],
                                         p1[0:64, :], sc[64:128, :])

                LOOKAHEAD = la
                for i, (wi, h) in enumerate(heads if not skip_heads else []):
                    stage_a(wi, h)
                    if i >= LOOKAHEAD:
                        stage_b(*heads[i - LOOKAHEAD])
                for j in (range(max(0, len(heads) - LOOKAHEAD), len(heads))
                          if not skip_heads else []):
                    stage_b(*heads[j])

                # O projection + residual -> X (pre-LN1), stats
                for mc in range(4):
                    po = psmm.tile([128, 288], F32, tag="mm")
                    for kc in range(4):
                        nc.tensor.matmul(po, lhsT=wo[:, kc, mc * 128:(mc + 1) * 128],
                                         rhs=ocm[:, kc, :], start=(kc == 0), stop=(kc == 3))
                    nc.vector.tensor_add(out=X[:, mc, cs0:cs0 + 288], in0=po,
                                         in1=X[:, mc, cs0:cs0 + 288])
                    nc.vector.tensor_add(out=X[:, mc, cs0:cs0 + 288],
                                         in0=X[:, mc, cs0:cs0 + 288],
                                         in1=bo[:, mc:mc + 1].broadcast_to([128, 288]))
                xsq = sqp.tile([128, 4, 288], BF16, tag="xsq")
                nc.vector.tensor_mul(xsq, xs, xs)
                ps_m = psaux.tile([1, 288], F32, tag="aux")
                for kc in range(4):
                    nc.tensor.matmul(ps_m, lhsT=onesf, rhs=xs[:, kc, :],
                                     start=(kc == 0), stop=(kc == 3))
                ps_s2 = psaux.tile([1, 288], F32, tag="aux")
                for kc in range(4):
                    nc.tensor.matmul(ps_s2, lhsT=ones, rhs=xsq[:, kc, :],
                                     start=(kc == 0), stop=(kc == 3))
                nc.scalar.activation(out=mrow[0:1, cs0:cs0 + 288], in_=ps_m,
                                     func=AF.Copy)
                nc.scalar.activation(out=mrow[0:1, T + cs0:T + cs0 + 288],
                                     in_=ps_s2, func=AF.Copy)

            # ---------------- LN helpers: half-batched rows + per-chunk apply
            HT = T // 2
            NSH = HT // 128

            def rows_half(half):
                hs = half * HT
                shb = lnp.tile([128, 2 * NSH], BF16, tag="shb")
                nc.sync.dma_start(out=shb[:, 0:NSH],
                                  in_=_insdim_ap(mrow[0:1, hs:hs + NSH], NSH, 128))
                nc.sync.dma_start(
                    out=shb[:, NSH:2 * NSH],
                    in_=_insdim_ap(mrow[0:1, T + hs:T + hs + NSH], NSH, 128))
                sh = lnp.tile([128, NSH], F32, tag="sh")
                msq = lnp.tile([128, NSH], F32, tag="msq")
                nc.vector.tensor_mul(msq, shb[:, 0:NSH], shb[:, 0:NSH])
                nc.vector.tensor_sub(sh, shb[:, NSH:2 * NSH], msq)
                nc.scalar.activation(out=sh, in_=sh, func=AF.Sqrt, bias=eps128)
                nc.vector.reciprocal(out=sh, in_=sh)
                shr = lnp.tile([128, NSH], BF16, tag="shr")
                nc.vector.tensor_copy(out=shr, in_=sh)
                nc.sync.dma_start(
                    out=_insdim_ap(mrow[0:1, T + hs:T + hs + NSH], NSH, 128),
                    in_=shr)

            def apply_chunk(cc, g, b, last=False):
                if True:
                    cs = cc * 512
                    mb = bcp.tile([128, 512], BF16, tag="mb")
                    nc.sync.dma_start(out=mb,
                                      in_=_bcast_ap(mrow[0:1, cs:cs + 512], 128))
                    rb = bcp.tile([128, 512], BF16, tag="rb")
                    nc.sync.dma_start(
                        out=rb, in_=_bcast_ap(mrow[0:1, T + cs:T + cs + 512], 128))
                    xc = X[:, :, cs:cs + 512]
                    nc.vector.tensor_sub(xc, xc,
                                         mb[:, None, :].broadcast_to([128, 4, 512]))
                    nc.vector.tensor_mul(xc, xc,
                                         rb[:, None, :].broadcast_to([128, 4, 512]))
                    ob = None
                    if last:
                        ob = sqp.tile([128, 4, 512], BF16, tag="xsq2", name="ob")
                    for mc in range(4):
                        dst = ob[:, mc, 0:512] if last else X[:, mc, cs:cs + 512]
                        nc.scalar.activation(out=dst, in_=X[:, mc, cs:cs + 512],
                                             func=AF.Identity, bias=b[:, mc:mc + 1],
                                             scale=g[:, mc:mc + 1])
                    if last:
                        nc.sync.dma_start(out=d["out"][:, :, cs:cs + 512], in_=ob)

            # ---------------- FFN chunk emitter --------------------------
            def ffn_chunk(cc):
                cs = cc * 512
                xc = X[:, :, cs:cs + 512]
                xb2 = xb2p.tile([128, 4, 512], BF16, tag="xb2")
                nc.gpsimd.tensor_copy(out=xb2, in_=xc)
                hb = hp.tile([128, 16, 512], BF16, tag="hb")
                for fc in range(16):
                    ph = psmm.tile([128, 512], F32, tag="mm")
                    for kc in range(4):
                        nc.tensor.matmul(ph, lhsT=w1[:, kc, fc * 128:(fc + 1) * 128],
                                         rhs=xb2[:, kc, :], start=(kc == 0), stop=(kc == 3))
                    nc.scalar.activation(out=hb[:, fc, :], in_=ph, func=AF.Relu,
                                         bias=bf1[:, fc:fc + 1])
                for mc in range(4):
                    pf = psmm.tile([128, 512], F32, tag="mm")
                    for fc in range(16):
                        nc.tensor.matmul(pf, lhsT=w2[:, fc, mc * 128:(mc + 1) * 128],
                                         rhs=hb[:, fc, :], start=(fc == 0), stop=(fc == 15))
                    nc.vector.tensor_add(out=X[:, mc, cs:cs + 512], in0=pf,
                                         in1=X[:, mc, cs:cs + 512])
                    nc.vector.tensor_add(out=X[:, mc, cs:cs + 512],
                                         in0=X[:, mc, cs:cs + 512],
                                         in1=bf2[:, mc:mc + 1].broadcast_to([128, 512]))
                xsq = sqp.tile([128, 4, 512], BF16, tag="xsq2")
                nc.vector.tensor_mul(xsq, xc, xc)
                ps_m = psaux.tile([1, 512], F32, tag="aux")
                for kc in range(4):
                    nc.tensor.matmul(ps_m, lhsT=onesf, rhs=xc[:, kc, :],
                                     start=(kc == 0), stop=(kc == 3))
                ps_s2 = psaux.tile([1, 512], F32, tag="aux")
                for kc in range(4):
                    nc.tensor.matmul(ps_s2, lhsT=ones, rhs=xsq[:, kc, :],
                                     start=(kc == 0), stop=(kc == 3))
                nc.scalar.activation(out=mrow[0:1, cs:cs + 512], in_=ps_m,
                                     func=AF.Copy)
                nc.scalar.activation(out=mrow[0:1, T + cs:T + cs + 512],
                                     in_=ps_s2, func=AF.Copy)

            # ---------------- layer schedule -----------------------------
            if not skip_attn:
                for p in range(PAIRS // 2):
                    att_pair(p)
                rows_half(0)
                for i, p in enumerate(range(PAIRS // 2, PAIRS)):
                    att_pair(p)
                    if i < 4:
                        apply_chunk(i, g1, b1)
                        if not skip_ffn:
                            ffn_chunk(i)
                lastl = (l == NL - 1)
                rows_half(1)
                for cc in range(4, NCH):
                    apply_chunk(cc, g1, b1)
                    if not skip_ffn:
                        ffn_chunk(cc)
                        if cc == 4:
                            # LN2 half-0 row math only needs ffn chunks 0..4 —
                            # emit it early so its latency hides under ffn 5..8.
                            rows_half(0)
                        if cc == 6:
                            # normalize chunk 0 early: unblocks the next
                            # layer's first attention pairs
                            apply_chunk(0, g2, b2, lastl)
            elif not skip_ffn:
                lastl = (l == NL - 1)
                for cc in range(NCH):
                    ffn_chunk(cc)
                    if cc == 4:
                        rows_half(0)
            if not skip_ffn:
                for cc in range(1, 4):
                    apply_chunk(cc, g2, b2, lastl)
                rows_half(1)
                for cc in range(4, NCH):
                    apply_chunk(cc, g2, b2, lastl)
            elif l == NL - 1:
                for cc in range(NCH):
                    cs = cc * 512
                    nc.sync.dma_start(out=d["out"][:, :, cs:cs + 512],
                                      in_=X[:, :, cs:cs + 512])

    return d


# ---------------------------------------------------------------------------
# Host-side packing + golden model
# ---------------------------------------------------------------------------

def rel_idx():
    coords = np.stack(np.meshgrid(np.arange(WS), np.arange(WS), indexing="ij"))
    flat = coords.reshape(2, -1)
    rel = (flat[:, :, None] - flat[:, None, :]).transpose(1, 2, 0).copy()
    rel[..., 0] += WS - 1
    rel[..., 1] += WS - 1
    rel[..., 0] *= 2 * WS - 1
    return rel.sum(-1)  # [N, N] int


def pack_weights(w, NL):
    """w: dict of reference arrays -> dict of const arrays (np)."""
    bf = ml_dtypes.bfloat16
    scale = HD ** -0.5
    ridx = rel_idx()
    out = {}

    def lhsT_pack(W, kchunks):  # [Cin, Cout] -> [128, kchunks, Cout]
        return np.ascontiguousarray(
            W.reshape(kchunks, 128, W.shape[1]).transpose(1, 0, 2)
        )

    wq = np.stack([lhsT_pack(w["Wq"][l] * scale, 4) for l in range(NL)])
    wk = np.stack([lhsT_pack(w["Wk"][l], 4) for l in range(NL)])
    wv = np.stack([lhsT_pack(w["Wv"][l], 4) for l in range(NL)])
    wo = np.stack([lhsT_pack(w["Wo"][l], 4) for l in range(NL)])
    w1 = np.stack([lhsT_pack(w["W1"][l], 4) for l in range(NL)])
    w2 = np.stack([lhsT_pack(w["W2"][l], 16) for l in range(NL)])
    for nm, arr in (("wq", wq), ("wk", wk), ("wv", wv), ("wo", wo),
                    ("w1", w1), ("w2", w2)):
        out[nm] = arr.astype(bf)

    expb = np.zeros((NL, 128, NH, 288), np.float32)
    for l in range(NL):
        bias = w["rpb"][l][ridx]            # [N(i), N(j), NH]
        ebT = np.exp(bias.transpose(2, 1, 0))  # [NH, j, i]
        expb[l, 0:128, :, 0:144] = ebT[:, 0:128, :].transpose(1, 0, 2)
        expb[l, 0:16, :, 144:288] = ebT[:, 128:144, :].transpose(1, 0, 2)
        expb[l, 32:48, :, 144:288] = ebT[:, 128:144, :].transpose(1, 0, 2)
    out["expb"] = expb.astype(bf)

    def percol(b):  # [NL, C] -> [NL, 128, 4]
        return np.ascontiguousarray(
            b.reshape(NL, 4, 128).transpose(0, 2, 1)).astype(np.float32)

    out["bq"] = percol(w["bq"] * scale)
    out["bk"] = percol(w["bk"])
    out["bo_c"] = percol(w["bo"])
    out["bf2_c"] = percol(w["bf2"])
    out["bo_r"] = w["bo"].reshape(NL, 1, 512).astype(bf)
    out["bf2_r"] = w["bf2"].reshape(NL, 1, 512).astype(bf)
    out["c_onesrow"] = np.ones((1, 512), bf)
    e2 = np.zeros((64, 128), np.float32)
    e2[0, 0:64] = 1.0
    e2[32, 64:128] = 1.0
    out["c_e2"] = e2.astype(bf)
    out["g1"] = percol(w["g1"])
    out["b1"] = percol(w["b1"])
    out["g2"] = percol(w["g2"])
    out["b2"] = percol(w["b2"])
    out["bf1"] = np.ascontiguousarray(
        w["bf1"].reshape(NL, 16, 128).transpose(0, 2, 1)).astype(np.float32)
    out["bvb"] = np.broadcast_to(
        w["bv"].astype(bf)[:, None, :], (NL, 128, 512)).copy()
    out["c_ones"] = np.full((128, 1), 1.0 / 512.0, bf)
    return out


def golden_tm(x_tm, w, NL):
    """fp32 numpy reference on window-major token-major x [T, 512]."""
    T = x_tm.shape[0]
    NW = T // N
    ridx = rel_idx()
    x = x_tm.astype(np.float32)

    def ln(v, g, b):
        m = v.mean(-1, keepdims=True)
        s = v.var(-1, keepdims=True)
        return (v - m) / np.sqrt(s + EPS) * g + b

    for l in range(NL):
        xw = x.reshape(NW, N, C)
        q = (xw @ w["Wq"][l] + w["bq"][l]).reshape(NW, N, NH, HD).transpose(0, 2, 1, 3)
        k = (xw @ w["Wk"][l] + w["bk"][l]).reshape(NW, N, NH, HD).transpose(0, 2, 1, 3)
        v = (xw @ w["Wv"][l] + w["bv"][l]).reshape(NW, N, NH, HD).transpose(0, 2, 1, 3)
        bias = w["rpb"][l][ridx].transpose(2, 0, 1)
        attn = np.einsum("whid,whjd->whij", q, k) * (HD ** -0.5) + bias
        attn = attn - attn.max(-1, keepdims=True)
        p = np.exp(attn)
        p = p / p.sum(-1, keepdims=True)
        o = np.einsum("whij,whjd->whid", p, v).transpose(0, 2, 1, 3).reshape(NW, N, C)
        o = o @ w["Wo"][l] + w["bo"][l]
        x = ln(o.reshape(T, C) + x, w["g1"][l], w["b1"][l])
        h = np.maximum(x @ w["W1"][l] + w["bf1"][l], 0.0) @ w["W2"][l] + w["bf2"][l]
        x = ln(h + x, w["g2"][l], w["b2"][l])
    return x


# ---------------------------------------------------------------------------
# kernel() entry point: full inputs -> full output, 8-way batch data parallel
# ---------------------------------------------------------------------------

NCORES = 8
B_FULL = 64
H_RES = W_RES = 24
L_TOK = H_RES * W_RES
NW_FULL = (B_FULL // NCORES) * (H_RES // WS) * (W_RES // WS)   # 32 windows/core
NL_FULL = 3

_COMPILED = {}


def _pack_x_all(x):
    """[64, 576, 512] f32 -> [8, 128, 4, T] bf16 channel-major window-major."""
    b = x.reshape(NCORES, B_FULL // NCORES, 2, WS, 2, WS, 4, 128)
    v = b.transpose(0, 7, 6, 1, 2, 4, 3, 5)   # [core,128,4, b,hw,ww,hs,ws]
    return np.ascontiguousarray(v.reshape(NCORES, 128, 4, -1)
                                ).astype(ml_dtypes.bfloat16)


def _unpack_out_all(res_list):
    """list of [128, 4, T] -> [64, 576, 512] f32."""
    y = np.stack([r.astype(np.float32) for r in res_list])     # [8,128,4,T]
    bpc = B_FULL // NCORES
    v = y.reshape(NCORES, 128, 4, bpc, 2, 2, WS, WS)
    v = v.transpose(0, 3, 4, 6, 5, 7, 2, 1)   # [core,b,hw,hs,ww,ws,4,128]
    return np.ascontiguousarray(v.reshape(B_FULL, L_TOK, C))


def kernel(x, Wq, bq, Wk, bk, Wv, bv, Wo, bo, rpb,
           g1, b1, W1, bf1, W2, bf2, g2, b2):
    import hashlib
    from concourse.bass_utils import run_bass_kernel_spmd

    w = {"Wq": np.asarray(Wq, np.float32), "bq": np.asarray(bq, np.float32),
         "Wk": np.asarray(Wk, np.float32), "bk": np.asarray(bk, np.float32),
         "Wv": np.asarray(Wv, np.float32), "bv": np.asarray(bv, np.float32),
         "Wo": np.asarray(Wo, np.float32), "bo": np.asarray(bo, np.float32),
         "rpb": np.asarray(rpb, np.float32),
         "g1": np.asarray(g1, np.float32), "b1": np.asarray(b1, np.float32),
         "W1": np.asarray(W1, np.float32), "bf1": np.asarray(bf1, np.float32),
         "W2": np.asarray(W2, np.float32), "bf2": np.asarray(bf2, np.float32),
         "g2": np.asarray(g2, np.float32), "b2": np.asarray(b2, np.float32)}
    hsh = hashlib.blake2b(
        b"".join(np.ascontiguousarray(v).tobytes() for v in w.values()),
        digest_size=16).hexdigest()
    if _COMPILED.get("hash") != hsh:
        packed = pack_weights(w, NL_FULL)
        nc = bacc.Bacc("TRN2", target_bir_lowering=False, debug=False)
        build(nc, NW_FULL, NL_FULL, packed)
        nc.compile()
        _COMPILED.update(hash=hsh, nc=nc)

    xp = _pack_x_all(np.asarray(x, np.float32))
    in_maps = [{"x": xp[i]} for i in range(NCORES)]
    res = run_bass_kernel_spmd(_COMPILED["nc"], in_maps, list(range(NCORES)))
    return _unpack_out_all([res.results[i]["out"] for i in range(NCORES)])

